# revision 5
# baseline (speedup 1.0000x reference)
"""AUGRU Trainium2 kernel — v7: seq-sorted shrinking widths, generic NCH.

Columns (batch rows) are sorted by sequence_length descending on the
host and dealt round-robin across the 8 cores x NCH chains, so every
chain sees the same width profile.  At step t only W_t =
ceil(alive(t)/(8*NCH)) columns are still live; every per-step
instruction's access pattern is sliced to W_t, shrinking the
width-dependent part of the serial cycle (ACT 0.83ns/col, DVE
0.52ns/col).  Outputs for dead columns are stale and masked on the
host.  The program is compiled per seq-len schedule (cached).

v7 additions over v6:
- NCH is a compile-time constant (2 or 3); batch padded to 8*NCH*CW.
- startup: first 2 steps' x/q DMA'd in a small transfer ahead of the
  block-0 bulk so step 0 isn't gated on the full block.
- NCH=3: bt product runs on GPSIMD to keep DVE under the cycle.
"""

import numpy as np

B, T, D, H = 2048, 200, 64, 64
NCORES = 8
NCH = 2                   # chains per core
CW = 128                  # columns per chain
BPAD = NCORES * NCH * CW  # padded batch
KBLK = 8

_CACHE = {}


def _build(ws):
    """ws: tuple of per-step live widths (len = number of steps)."""
    key = ("nc", NCH, CW, ws)
    if key in _CACHE:
        return _CACHE[key]

    from contextlib import ExitStack
    import concourse.tile as tile
    from concourse import bacc, mybir

    TS = len(ws)
    CBLK = (TS - 1) // KBLK         # last block with compute steps

    f32 = mybir.dt.float32
    bf16 = mybir.dt.float16
    ALU = mybir.AluOpType
    AF = mybir.ActivationFunctionType

    nc = bacc.Bacc("TRN2", target_bir_lowering=False, debug=False,
                   enable_asserts=False, num_devices=NCORES)

    XD = nc.dram_tensor("x", [NCH, D, T, CW], bf16, kind="ExternalInput").ap()
    QD = nc.dram_tensor("q", [NCH, H, T, CW], bf16, kind="ExternalInput").ap()
    WPK = nc.dram_tensor("wpk", [D, 2 * H + 2 * H + H + H], bf16, kind="ExternalInput").ap()
    BPK = nc.dram_tensor("bpk", [2 * H, 3], f32, kind="ExternalInput").ap()
    OUT = nc.dram_tensor("out", [NCH, H, T, CW], bf16, kind="ExternalOutput").ap()

    with tile.TileContext(nc) as tc:
        with ExitStack() as ctx:
            consts = ctx.enter_context(tc.tile_pool(name="consts", bufs=1))
            state = ctx.enter_context(tc.tile_pool(name="state", bufs=1))
            gpoolG = ctx.enter_context(tc.tile_pool(name="gatesG", bufs=165))
            gpoolS = ctx.enter_context(tc.tile_pool(name="gatesS", bufs=165))
            tpool = ctx.enter_context(tc.tile_pool(name="tmp", bufs=6))
            rpool = ctx.enter_context(tc.tile_pool(name="rh", bufs=8))
            ps_zg = ctx.enter_context(tc.tile_pool(name="zg", bufs=2, space="PSUM"))
            ps_zc = ctx.enter_context(tc.tile_pool(name="zc", bufs=2, space="PSUM"))

            # ---- shared constants ----
            wpk_sb = consts.tile([D, 6 * H], bf16, tag="wpk")
            nc.sync.dma_start(out=wpk_sb[:], in_=WPK[:])
            bpk_sb = consts.tile([2 * H, 3], f32, tag="bpk")

            # ---- per-chain staged/rotating tiles ----
            xst = [[state.tile([D, KBLK, CW], bf16, tag=f"xst{c}_{j}", name=f"xst{c}_{j}")
                    for j in range(2)] for c in range(NCH)]
            hq = [[state.tile([2 * H, KBLK, CW], bf16, tag=f"hq{c}_{j}", name=f"hq{c}_{j}")
                    for j in range(2)] for c in range(NCH)]

            # step-0/1 x and q land first in small transfers so the first
            # gate matmul isn't gated on the full block-0 DMA
            for c in range(NCH):
                nc.sync.dma_start(out=xst[c][0][:, 0:2, :], in_=XD[c, :, 0:2, :])
                nc.sync.dma_start(out=hq[c][0][H:, 0:2, :], in_=QD[c, :, 0:2, :])
            nc.sync.dma_start(out=bpk_sb[:], in_=BPK[:])
            for c in range(NCH):
                nc.sync.dma_start(out=xst[c][0][:, 2:KBLK, :], in_=XD[c, :, 2:KBLK, :])
                nc.sync.dma_start(out=hq[c][0][H:, 2:KBLK, :], in_=QD[c, :, 2:KBLK, :])

            w1x_sb = wpk_sb[:, 0:2 * H]
            w1h_sb = wpk_sb[:, 2 * H:4 * H]
            w2x_sb = wpk_sb[:, 4 * H:5 * H]
            w2h_sb = wpk_sb[:, 5 * H:6 * H]
            bg_sb = bpk_sb[:, 0:1]
            bc_sb = bpk_sb[0:H, 1:2]
            # dummy sigmoid: hoists the ACT table load off the first chain step
            scr = consts.tile([1, 2], bf16, tag="scr")
            scr3 = consts.tile([1, 2], bf16, tag="scr3")
            nc.scalar.activation(scr[:], bpk_sb[0:1, 0:2], AF.Sigmoid)

            ones_hi = [consts.tile([2 * H, CW], bf16, tag=f"oneshi{c}", name=f"oneshi{c}") for c in range(NCH)]
            half_bt = [consts.tile([H, CW], bf16, tag=f"halfbt{c}", name=f"halfbt{c}") for c in range(NCH)]
            half_m = [consts.tile([H, CW], bf16, tag=f"halfm{c}", name=f"halfm{c}") for c in range(NCH)]
            for c in range(NCH):
                nc.gpsimd.memset(ones_hi[c][H:, :], 1.0)
                nc.vector.memset(half_bt[c][:], 0.5)
                nc.vector.memset(half_m[c][:], 0.5)

            hst = [state.tile([2 * H, 2, CW], bf16, tag=f"hst{c}", name=f"hst{c}")
                   for c in range(NCH)]
            btt = [[state.tile([H, CW], bf16, tag=f"btt{c}_{j}", name=f"btt{c}_{j}")
                    for j in range(2)] for c in range(NCH)]
            mtt = [[state.tile([H, CW], bf16, tag=f"mtt{c}_{j}", name=f"mtt{c}_{j}")
                    for j in range(4)] for c in range(NCH)]

            for c in range(NCH):
                nc.vector.memset(hq[c][0][0:H, 0, :], 0.0)

            # DMA/OUT slot schedule within each 8-step block, per chain
            x_slot = {c: c for c in range(NCH)}                  # prefetch x blk+1
            q_slot = {c: (3 + c if NCH == 3 else 2 * c + 1) for c in range(NCH)}
            o_slot = {c: (5 + c if NCH == 3 else 4 + 2 * c) for c in range(NCH)}
            bt_engine = nc.gpsimd if NCH >= 3 else nc.vector

            last_out_j = [-1 for _ in range(NCH)]   # last OUT block written per chain

            for t in range(TS):
                W = ws[t]
                blk, ci = divmod(t, KBLK)
                pb = blk % 2
                for c in range(NCH):
                    # spread the block DMAs across the block's steps
                    if ci == x_slot[c] and blk + 1 <= CBLK:
                        nc.sync.dma_start(out=xst[c][(blk + 1) % 2][:, :, :],
                                          in_=XD[c, :, (blk + 1) * KBLK:(blk + 2) * KBLK, :])
                    if ci == q_slot[c] and blk + 1 <= CBLK:
                        nc.sync.dma_start(out=hq[c][(blk + 1) % 2][H:, :, :],
                                          in_=QD[c, :, (blk + 1) * KBLK:(blk + 2) * KBLK, :])
                    if ci == 6 and blk + 1 <= CBLK:
                        scr2 = tpool.tile([1, 2], bf16, tag=f"scr2{c}", name=f"scr2{c}")
                        nc.vector.tensor_copy(scr2[:], hq[c][(blk + 1) % 2][H:H + 1, 0, 0:2])
                    if ci == o_slot[c] and blk >= 1:
                        j = blk - 1
                        nc.sync.dma_start(
                            out=OUT[c, :, j * KBLK:(j + 1) * KBLK - 1, :],
                            in_=hq[c][j % 2][0:H, 1:KBLK, :])
                        nc.sync.dma_start(
                            out=OUT[c, :, (j + 1) * KBLK - 1:(j + 1) * KBLK, :],
                            in_=hq[c][(j + 1) % 2][0:H, 0:1, :])
                        last_out_j[c] = j

                    if t == 0:
                        h_tilde_prev = ones_hi[c][H:, 0:W]
                        bt_prev = half_bt[c][:, 0:W]
                        m_prev = half_m[c][:, 0:W]
                    else:
                        h_tilde_prev = hst[c][H:, (t - 1) % 2, 0:W]
                        bt_prev = btt[c][(t - 1) % 2][:, 0:W]
                        m_prev = mtt[c][(t - 1) % 4][:, 0:W]

                    # gate preactivation
                    zg = ps_zg.tile([2 * H, CW], f32, tag=f"zg{c}", name=f"zg{c}")
                    nc.tensor.matmul(zg[:, 0:W], lhsT=w1x_sb,
                                     rhs=xst[c][pb][:, ci, 0:W],
                                     start=True, stop=(t == 0))
                    if t > 0:
                        nc.tensor.matmul(zg[:, 0:W], lhsT=w1h_sb, rhs=bt_prev,
                                         start=False, stop=False)
                        nc.tensor.matmul(zg[:, 0:W], lhsT=w1h_sb, rhs=m_prev,
                                         start=False, stop=True)

                    G = gpoolG.tile([2 * H, CW], bf16, tag=f"G{c}", name=f"G{c}")
                    nc.scalar.activation(G[:, 0:W], zg[:, 0:W], AF.Sigmoid,
                                         bias=bg_sb if t > 0 else bpk_sb[:, 2:3])

                    # DVE: rh (chain), uh, a2; bt on Pool when NCH>=3
                    ru = rpool.tile([2 * H, CW], bf16, tag=f"ru{c}", name=f"ru{c}")
                    nc.vector.tensor_tensor(ru[:, 0:W], hq[c][pb][:, ci, 0:W],
                                            G[:, 0:W], op=ALU.mult)
                    a2 = tpool.tile([H, CW], bf16, tag=f"a2{c}", name=f"a2{c}")
                    nc.vector.tensor_scalar(a2[:, 0:W], ru[H:, 0:W], -2.0, 2.0,
                                            op0=ALU.mult, op1=ALU.add)
                    bt_cur = btt[c][t % 2]
                    bt_engine.tensor_tensor(bt_cur[:, 0:W], ru[H:, 0:W], h_tilde_prev,
                                            op=ALU.mult)

                    # candidate
                    zc = ps_zc.tile([H, CW], f32, tag=f"zc{c}", name=f"zc{c}")
                    nc.tensor.matmul(zc[:, 0:W], lhsT=w2x_sb,
                                     rhs=xst[c][pb][:, ci, 0:W],
                                     start=True, stop=(t == 0))
                    if t > 0:
                        nc.tensor.matmul(zc[:, 0:W], lhsT=w2h_sb, rhs=ru[0:H, 0:W],
                                         start=False, stop=True)
                    s = gpoolS.tile([H, CW], bf16, tag=f"s{c}", name=f"s{c}")
                    nc.scalar.activation(s[:, 0:W], zc[:, 0:W], AF.Sigmoid, bias=bc_sb,
                                         scale=2.0)

                    # m (chain, DVE); h~' and h' follow
                    m_cur = mtt[c][t % 4]
                    nc.vector.tensor_tensor(m_cur[:, 0:W], a2[:, 0:W], s[:, 0:W],
                                            op=ALU.mult)
                    nc.vector.tensor_tensor(hst[c][H:, t % 2, 0:W], bt_cur[:, 0:W],
                                            m_cur[:, 0:W], op=ALU.add)
                    nblk2, nci = divmod(t + 1, KBLK)
                    nc.vector.tensor_scalar(hq[c][nblk2 % 2][0:H, nci, 0:W],
                                            hst[c][H:, t % 2, 0:W],
                                            -1.0, None, op0=ALU.add)
                    if c == NCH - 1 and t >= TS - 44 and t % 8 == 5:
                        # advance ACT's DVE-sem frontier: sigmoid WAR conds
                        # on reused G/s tiles get statically subsumed
                        nc.scalar.activation(scr3[:], mtt[NCH - 1][(t - 2) % 4][0:1, 0:2],
                                             AF.Sigmoid)

            # Epilogue: flush OUT blocks not yet written.  h_p lives in
            # hq[(p+1)//8 % 2] slot (p+1)%8; positions p = 0..TS-1.
            for c in range(NCH):
                for j in range(last_out_j[c] + 1, CBLK + 1):
                    hi_p = min((j + 1) * KBLK - 2, TS - 1)       # last p with slot in block j
                    nslots = hi_p - j * KBLK + 1
                    if nslots >= 1:
                        nc.sync.dma_start(
                            out=OUT[c, :, j * KBLK:j * KBLK + nslots, :],
                            in_=hq[c][j % 2][0:H, 1:1 + nslots, :])
                    p = (j + 1) * KBLK - 1                        # slot 0 of block j+1
                    if p <= TS - 1:
                        nc.sync.dma_start(
                            out=OUT[c, :, p:p + 1, :],
                            in_=hq[c][(j + 1) % 2][0:H, 0:1, :])

    nc.compile()
    _CACHE[key] = nc
    return nc


def _prep_shared(gate_kernel, gate_bias, cand_kernel, cand_bias):
    bf = np.dtype(np.float16)
    gk = np.asarray(gate_kernel, np.float32)
    gb = np.asarray(gate_bias, np.float32)
    ck = np.asarray(cand_kernel, np.float32)
    cb = np.asarray(cand_bias, np.float32)
    wpk = np.concatenate([gk[:D], gk[D:], ck[:D], ck[D:]], axis=1).astype(bf)  # [64, 384]
    bg = (gb - gk[D:].sum(axis=0)).astype(np.float32)
    bpk = np.zeros((2 * H, 3), np.float32)
    bpk[:, 0] = bg
    bpk[:H, 1] = 2.0 * cb
    bpk[:, 2] = gb
    return np.ascontiguousarray(wpk), bpk


def _schedule(seq_len):
    """Sorted column assignment + per-step width schedule.

    Returns (cols, ws): cols[k, c, i] = padded-batch row for core k,
    chain c, column i (rows >= B are zero padding);  ws[t] = compiled
    live width at step t.
    """
    seq = np.zeros(BPAD, np.int64)
    seq[:B] = np.asarray(seq_len, np.int64)
    order = np.argsort(-seq, kind="stable")          # rank r -> padded row
    cols = np.empty((NCORES, NCH, CW), np.int64)
    for k in range(NCORES):
        for c in range(NCH):
            cols[k, c] = order[k + NCORES * c + NCORES * NCH * np.arange(CW)]
    sdesc = seq[order]
    ts = max(int(sdesc[0]), 1)                       # steps needed = max seq_len
    alive = np.array([(sdesc > t).sum() for t in range(ts)], np.int64)
    ws = np.minimum(CW, np.maximum(2, -(-alive // (NCORES * NCH))))
    return cols, tuple(int(w) for w in ws)


def _run(inputs, trace=False):
    from concourse.bass_utils import run_bass_kernel_spmd

    bf = np.dtype(np.float16)
    rnn_input = np.asarray(inputs["rnn_input"], np.float32)
    seq_len = np.asarray(inputs["sequence_length"], np.int32)
    att = np.asarray(inputs["att_score"], np.float32)
    wpk, bpk = _prep_shared(
        inputs["gate_kernel"], inputs["gate_bias"],
        inputs["cand_kernel"], inputs["cand_bias"])

    cols, ws = _schedule(seq_len)
    nc = _build(ws)

    rnn_pad = np.zeros((BPAD, T, D), np.float32)
    rnn_pad[:B] = rnn_input
    q_pad = np.ones((BPAD, T), np.float32)
    q_pad[:B] = 1.0 - att[:, :, 0]

    in_maps = []
    for k in range(NCORES):
        xi = np.empty((NCH, D, T, CW), bf)
        qi = np.empty((NCH, H, T, CW), bf)
        for c in range(NCH):
            rows = cols[k, c]
            xi[c] = rnn_pad[rows].transpose(2, 1, 0).astype(bf)
            q = q_pad[rows].T.astype(np.float32)               # [T,CW]
            qi[c] = np.broadcast_to(q[None], (H, T, CW)).astype(bf)
        in_maps.append({"x": np.ascontiguousarray(xi),
                        "q": np.ascontiguousarray(qi),
                        "wpk": wpk, "bpk": bpk})

    res = run_bass_kernel_spmd(nc, in_maps, core_ids=list(range(NCORES)), trace=trace)

    out = np.zeros((B, T, H), np.float32)
    t_idx = np.arange(T, dtype=np.int32)
    for k in range(NCORES):
        y = np.asarray(res.results[k]["out"], np.float32)   # [NCH,H,T,CW]
        for c in range(NCH):
            rows = cols[k, c]
            live = rows < B
            r = rows[live]
            hi = y[c].transpose(2, 1, 0)[live]              # [nlive,T,H]
            mask = (t_idx[None, :] < seq_len[r, None]).astype(np.float32)
            out[r] = hi * mask[:, :, None]
    return out, res


def kernel(**inputs):
    out, _ = _run(inputs)
    return out


# revision 7
# speedup vs baseline: 1.0023x; 1.0023x over previous
"""AUGRU Trainium2 kernel — v7: seq-sorted shrinking widths, generic NCH.

Columns (batch rows) are sorted by sequence_length descending on the
host and dealt round-robin across the 8 cores x NCH chains, so every
chain sees the same width profile.  At step t only W_t =
ceil(alive(t)/(8*NCH)) columns are still live; every per-step
instruction's access pattern is sliced to W_t, shrinking the
width-dependent part of the serial cycle (ACT 0.83ns/col, DVE
0.52ns/col).  Outputs for dead columns are stale and masked on the
host.  The program is compiled per seq-len schedule (cached).

v7 additions over v6:
- NCH is a compile-time constant (2 or 3); batch padded to 8*NCH*CW.
- startup: first 2 steps' x/q DMA'd in a small transfer ahead of the
  block-0 bulk so step 0 isn't gated on the full block.
- NCH=3: bt product runs on GPSIMD to keep DVE under the cycle.
"""

import numpy as np

B, T, D, H = 2048, 200, 64, 64
NCORES = 8
NCH = 2                   # chains per core
CW = 128                  # columns per chain
BPAD = NCORES * NCH * CW  # padded batch
KBLK = 8

_CACHE = {}


def _build(ws):
    """ws: tuple of per-step live widths (len = number of steps)."""
    key = ("nc", NCH, CW, ws)
    if key in _CACHE:
        return _CACHE[key]

    from contextlib import ExitStack
    import concourse.tile as tile
    from concourse import bacc, mybir

    TS = len(ws)
    CBLK = (TS - 1) // KBLK         # last block with compute steps

    f32 = mybir.dt.float32
    bf16 = mybir.dt.float16
    ALU = mybir.AluOpType
    AF = mybir.ActivationFunctionType

    nc = bacc.Bacc("TRN2", target_bir_lowering=False, debug=False,
                   enable_asserts=False, num_devices=NCORES)

    XD = nc.dram_tensor("x", [NCH, D, T, CW], bf16, kind="ExternalInput").ap()
    QD = nc.dram_tensor("q", [NCH, H, T, CW], bf16, kind="ExternalInput").ap()
    WPK = nc.dram_tensor("wpk", [D, 2 * H + 2 * H + H + H], bf16, kind="ExternalInput").ap()
    BPK = nc.dram_tensor("bpk", [2 * H, 3], f32, kind="ExternalInput").ap()
    OUT = nc.dram_tensor("out", [NCH, H, T, CW], bf16, kind="ExternalOutput").ap()

    with tile.TileContext(nc) as tc:
        with ExitStack() as ctx:
            consts = ctx.enter_context(tc.tile_pool(name="consts", bufs=1))
            state = ctx.enter_context(tc.tile_pool(name="state", bufs=1))
            gpoolG = ctx.enter_context(tc.tile_pool(name="gatesG", bufs=165))
            gpoolS = ctx.enter_context(tc.tile_pool(name="gatesS", bufs=165))
            tpool = ctx.enter_context(tc.tile_pool(name="tmp", bufs=6))
            rpool = ctx.enter_context(tc.tile_pool(name="rh", bufs=8))
            ps_zg = ctx.enter_context(tc.tile_pool(name="zg", bufs=2, space="PSUM"))
            ps_zc = ctx.enter_context(tc.tile_pool(name="zc", bufs=2, space="PSUM"))

            # ---- shared constants ----
            wpk_sb = consts.tile([D, 6 * H], bf16, tag="wpk")
            nc.sync.dma_start(out=wpk_sb[:], in_=WPK[:])
            bpk_sb = consts.tile([2 * H, 3], f32, tag="bpk")

            # ---- per-chain staged/rotating tiles ----
            xst = [[state.tile([D, KBLK, CW], bf16, tag=f"xst{c}_{j}", name=f"xst{c}_{j}")
                    for j in range(2)] for c in range(NCH)]
            hq = [[state.tile([2 * H, KBLK, CW], bf16, tag=f"hq{c}_{j}", name=f"hq{c}_{j}")
                    for j in range(2)] for c in range(NCH)]

            # step-0/1 x and q land first in small transfers so the first
            # gate matmul isn't gated on the full block-0 DMA
            nc.sync.dma_start(out=xst[0][0][:, 0:2, :], in_=XD[0, :, 0:2, :])
            nc.sync.dma_start(out=bpk_sb[:], in_=BPK[:])
            nc.sync.dma_start(out=hq[0][0][H:, 0:2, :], in_=QD[0, :, 0:2, :])
            for c in range(1, NCH):
                nc.sync.dma_start(out=xst[c][0][:, 0:2, :], in_=XD[c, :, 0:2, :])
                nc.sync.dma_start(out=hq[c][0][H:, 0:2, :], in_=QD[c, :, 0:2, :])
            for c in range(NCH):
                nc.sync.dma_start(out=xst[c][0][:, 2:KBLK, :], in_=XD[c, :, 2:KBLK, :])
                nc.sync.dma_start(out=hq[c][0][H:, 2:KBLK, :], in_=QD[c, :, 2:KBLK, :])

            w1x_sb = wpk_sb[:, 0:2 * H]
            w1h_sb = wpk_sb[:, 2 * H:4 * H]
            w2x_sb = wpk_sb[:, 4 * H:5 * H]
            w2h_sb = wpk_sb[:, 5 * H:6 * H]
            bg_sb = bpk_sb[:, 0:1]
            bc_sb = bpk_sb[0:H, 1:2]
            # dummy sigmoid: hoists the ACT table load off the first chain step
            scr = consts.tile([1, 2], bf16, tag="scr")
            scr3 = consts.tile([1, 2], bf16, tag="scr3")
            nc.scalar.activation(scr[:], bpk_sb[0:1, 0:2], AF.Sigmoid)

            ones_hi = [consts.tile([2 * H, CW], bf16, tag=f"oneshi{c}", name=f"oneshi{c}") for c in range(NCH)]
            half_bt = [consts.tile([H, CW], bf16, tag=f"halfbt{c}", name=f"halfbt{c}") for c in range(NCH)]
            half_m = [consts.tile([H, CW], bf16, tag=f"halfm{c}", name=f"halfm{c}") for c in range(NCH)]
            for c in range(NCH):
                nc.gpsimd.memset(ones_hi[c][H:, :], 1.0)
                nc.vector.memset(half_bt[c][:], 0.5)
                nc.vector.memset(half_m[c][:], 0.5)

            hst = [state.tile([2 * H, 2, CW], bf16, tag=f"hst{c}", name=f"hst{c}")
                   for c in range(NCH)]
            btt = [[state.tile([H, CW], bf16, tag=f"btt{c}_{j}", name=f"btt{c}_{j}")
                    for j in range(2)] for c in range(NCH)]
            mtt = [[state.tile([H, CW], bf16, tag=f"mtt{c}_{j}", name=f"mtt{c}_{j}")
                    for j in range(4)] for c in range(NCH)]

            for c in range(NCH):
                nc.vector.memset(hq[c][0][0:H, 0, :], 0.0)

            # DMA/OUT slot schedule within each 8-step block, per chain
            if NCH == 3:
                x_slot = {0: 0, 1: 1, 2: 2}                      # prefetch x blk+1
                q_slot = {0: 3, 1: 4, 2: 5}
                o_slot = {0: 6, 1: 7, 2: 5}
            else:
                x_slot = {c: 2 * c for c in range(NCH)}
                q_slot = {c: 2 * c + 1 for c in range(NCH)}
                o_slot = {c: 4 + 2 * c for c in range(NCH)}
            bt_engine = nc.gpsimd if NCH >= 3 else nc.vector

            last_out_j = [-1 for _ in range(NCH)]   # last OUT block written per chain

            for t in range(TS):
                W = ws[t]
                blk, ci = divmod(t, KBLK)
                pb = blk % 2
                for c in range(NCH):
                    # spread the block DMAs across the block's steps
                    if ci == x_slot[c] and blk + 1 <= CBLK:
                        nc.sync.dma_start(out=xst[c][(blk + 1) % 2][:, :, :],
                                          in_=XD[c, :, (blk + 1) * KBLK:(blk + 2) * KBLK, :])
                    if ci == q_slot[c] and blk + 1 <= CBLK:
                        nc.sync.dma_start(out=hq[c][(blk + 1) % 2][H:, :, :],
                                          in_=QD[c, :, (blk + 1) * KBLK:(blk + 2) * KBLK, :])
                    if ci == 6 and blk + 1 <= CBLK:
                        scr2 = tpool.tile([1, 2], bf16, tag=f"scr2{c}", name=f"scr2{c}")
                        nc.vector.tensor_copy(scr2[:], hq[c][(blk + 1) % 2][H:H + 1, 0, 0:2])
                    if ci == o_slot[c] and blk >= 1:
                        j = blk - 1
                        nc.sync.dma_start(
                            out=OUT[c, :, j * KBLK:(j + 1) * KBLK - 1, :],
                            in_=hq[c][j % 2][0:H, 1:KBLK, :])
                        nc.sync.dma_start(
                            out=OUT[c, :, (j + 1) * KBLK - 1:(j + 1) * KBLK, :],
                            in_=hq[c][(j + 1) % 2][0:H, 0:1, :])
                        last_out_j[c] = j

                    if t == 0:
                        h_tilde_prev = ones_hi[c][H:, 0:W]
                        bt_prev = half_bt[c][:, 0:W]
                        m_prev = half_m[c][:, 0:W]
                    else:
                        h_tilde_prev = hst[c][H:, (t - 1) % 2, 0:W]
                        bt_prev = btt[c][(t - 1) % 2][:, 0:W]
                        m_prev = mtt[c][(t - 1) % 4][:, 0:W]

                    # gate preactivation
                    zg = ps_zg.tile([2 * H, CW], f32, tag=f"zg{c}", name=f"zg{c}")
                    nc.tensor.matmul(zg[:, 0:W], lhsT=w1x_sb,
                                     rhs=xst[c][pb][:, ci, 0:W],
                                     start=True, stop=(t == 0))
                    if t > 0:
                        nc.tensor.matmul(zg[:, 0:W], lhsT=w1h_sb, rhs=bt_prev,
                                         start=False, stop=False)
                        nc.tensor.matmul(zg[:, 0:W], lhsT=w1h_sb, rhs=m_prev,
                                         start=False, stop=True)

                    G = gpoolG.tile([2 * H, CW], bf16, tag=f"G{c}", name=f"G{c}")
                    nc.scalar.activation(G[:, 0:W], zg[:, 0:W], AF.Sigmoid,
                                         bias=bg_sb if t > 0 else bpk_sb[:, 2:3])

                    # DVE: rh (chain), uh, a2; bt on Pool when NCH>=3
                    ru = rpool.tile([2 * H, CW], bf16, tag=f"ru{c}", name=f"ru{c}")
                    nc.vector.tensor_tensor(ru[:, 0:W], hq[c][pb][:, ci, 0:W],
                                            G[:, 0:W], op=ALU.mult)
                    a2 = tpool.tile([H, CW], bf16, tag=f"a2{c}", name=f"a2{c}")
                    nc.vector.tensor_scalar(a2[:, 0:W], ru[H:, 0:W], -2.0, 2.0,
                                            op0=ALU.mult, op1=ALU.add)
                    bt_cur = btt[c][t % 2]
                    bt_engine.tensor_tensor(bt_cur[:, 0:W], ru[H:, 0:W], h_tilde_prev,
                                            op=ALU.mult)

                    # candidate
                    zc = ps_zc.tile([H, CW], f32, tag=f"zc{c}", name=f"zc{c}")
                    nc.tensor.matmul(zc[:, 0:W], lhsT=w2x_sb,
                                     rhs=xst[c][pb][:, ci, 0:W],
                                     start=True, stop=(t == 0))
                    if t > 0:
                        nc.tensor.matmul(zc[:, 0:W], lhsT=w2h_sb, rhs=ru[0:H, 0:W],
                                         start=False, stop=True)
                    s = gpoolS.tile([H, CW], bf16, tag=f"s{c}", name=f"s{c}")
                    nc.scalar.activation(s[:, 0:W], zc[:, 0:W], AF.Sigmoid, bias=bc_sb,
                                         scale=2.0)

                    # m (chain, DVE); h~' and h' follow
                    m_cur = mtt[c][t % 4]
                    nc.vector.tensor_tensor(m_cur[:, 0:W], a2[:, 0:W], s[:, 0:W],
                                            op=ALU.mult)
                    nc.vector.tensor_tensor(hst[c][H:, t % 2, 0:W], bt_cur[:, 0:W],
                                            m_cur[:, 0:W], op=ALU.add)
                    nblk2, nci = divmod(t + 1, KBLK)
                    nc.vector.tensor_scalar(hq[c][nblk2 % 2][0:H, nci, 0:W],
                                            hst[c][H:, t % 2, 0:W],
                                            -1.0, None, op0=ALU.add)
                    if c == NCH - 1 and t >= TS - 44 and t % 8 == 5:
                        # advance ACT's DVE-sem frontier: sigmoid WAR conds
                        # on reused G/s tiles get statically subsumed
                        nc.scalar.activation(scr3[:], mtt[NCH - 1][(t - 2) % 4][0:1, 0:2],
                                             AF.Sigmoid)

            # Epilogue: flush OUT blocks not yet written.  h_p lives in
            # hq[(p+1)//8 % 2] slot (p+1)%8; positions p = 0..TS-1.
            for c in range(NCH):
                for j in range(last_out_j[c] + 1, CBLK + 1):
                    hi_p = min((j + 1) * KBLK - 2, TS - 1)       # last p with slot in block j
                    nslots = hi_p - j * KBLK + 1
                    if nslots >= 1:
                        nc.sync.dma_start(
                            out=OUT[c, :, j * KBLK:j * KBLK + nslots, :],
                            in_=hq[c][j % 2][0:H, 1:1 + nslots, :])
                    p = (j + 1) * KBLK - 1                        # slot 0 of block j+1
                    if p <= TS - 1:
                        nc.sync.dma_start(
                            out=OUT[c, :, p:p + 1, :],
                            in_=hq[c][(j + 1) % 2][0:H, 0:1, :])

    nc.compile()
    _CACHE[key] = nc
    return nc


def _prep_shared(gate_kernel, gate_bias, cand_kernel, cand_bias):
    bf = np.dtype(np.float16)
    gk = np.asarray(gate_kernel, np.float32)
    gb = np.asarray(gate_bias, np.float32)
    ck = np.asarray(cand_kernel, np.float32)
    cb = np.asarray(cand_bias, np.float32)
    wpk = np.concatenate([gk[:D], gk[D:], ck[:D], ck[D:]], axis=1).astype(bf)  # [64, 384]
    bg = (gb - gk[D:].sum(axis=0)).astype(np.float32)
    bpk = np.zeros((2 * H, 3), np.float32)
    bpk[:, 0] = bg
    bpk[:H, 1] = 2.0 * cb
    bpk[:, 2] = gb
    return np.ascontiguousarray(wpk), bpk


def _schedule(seq_len):
    """Sorted column assignment + per-step width schedule.

    Returns (cols, ws): cols[k, c, i] = padded-batch row for core k,
    chain c, column i (rows >= B are zero padding);  ws[t] = compiled
    live width at step t.
    """
    seq = np.zeros(BPAD, np.int64)
    seq[:B] = np.asarray(seq_len, np.int64)
    order = np.argsort(-seq, kind="stable")          # rank r -> padded row
    cols = np.empty((NCORES, NCH, CW), np.int64)
    for k in range(NCORES):
        for c in range(NCH):
            cols[k, c] = order[k + NCORES * c + NCORES * NCH * np.arange(CW)]
    sdesc = seq[order]
    ts = max(int(sdesc[0]), 1)                       # steps needed = max seq_len
    alive = np.array([(sdesc > t).sum() for t in range(ts)], np.int64)
    ws = np.minimum(CW, np.maximum(2, -(-alive // (NCORES * NCH))))
    return cols, tuple(int(w) for w in ws)


def _run(inputs, trace=False):
    from concourse.bass_utils import run_bass_kernel_spmd

    bf = np.dtype(np.float16)
    rnn_input = np.asarray(inputs["rnn_input"], np.float32)
    seq_len = np.asarray(inputs["sequence_length"], np.int32)
    att = np.asarray(inputs["att_score"], np.float32)
    wpk, bpk = _prep_shared(
        inputs["gate_kernel"], inputs["gate_bias"],
        inputs["cand_kernel"], inputs["cand_bias"])

    cols, ws = _schedule(seq_len)
    nc = _build(ws)

    rnn_pad = np.zeros((BPAD, T, D), np.float32)
    rnn_pad[:B] = rnn_input
    q_pad = np.ones((BPAD, T), np.float32)
    q_pad[:B] = 1.0 - att[:, :, 0]

    in_maps = []
    for k in range(NCORES):
        xi = np.empty((NCH, D, T, CW), bf)
        qi = np.empty((NCH, H, T, CW), bf)
        for c in range(NCH):
            rows = cols[k, c]
            xi[c] = rnn_pad[rows].transpose(2, 1, 0).astype(bf)
            q = q_pad[rows].T.astype(np.float32)               # [T,CW]
            qi[c] = np.broadcast_to(q[None], (H, T, CW)).astype(bf)
        in_maps.append({"x": np.ascontiguousarray(xi),
                        "q": np.ascontiguousarray(qi),
                        "wpk": wpk, "bpk": bpk})

    res = run_bass_kernel_spmd(nc, in_maps, core_ids=list(range(NCORES)), trace=trace)

    out = np.zeros((B, T, H), np.float32)
    t_idx = np.arange(T, dtype=np.int32)
    for k in range(NCORES):
        y = np.asarray(res.results[k]["out"], np.float32)   # [NCH,H,T,CW]
        for c in range(NCH):
            rows = cols[k, c]
            live = rows < B
            r = rows[live]
            hi = y[c].transpose(2, 1, 0)[live]              # [nlive,T,H]
            mask = (t_idx[None, :] < seq_len[r, None]).astype(np.float32)
            out[r] = hi * mask[:, :, None]
    return out, res


def kernel(**inputs):
    out, _ = _run(inputs)
    return out


# revision 9
# speedup vs baseline: 1.0076x; 1.0053x over previous
"""AUGRU Trainium2 kernel — v7: seq-sorted shrinking widths, generic NCH.

Columns (batch rows) are sorted by sequence_length descending on the
host and dealt round-robin across the 8 cores x NCH chains, so every
chain sees the same width profile.  At step t only W_t =
ceil(alive(t)/(8*NCH)) columns are still live; every per-step
instruction's access pattern is sliced to W_t, shrinking the
width-dependent part of the serial cycle (ACT 0.83ns/col, DVE
0.52ns/col).  Outputs for dead columns are stale and masked on the
host.  The program is compiled per seq-len schedule (cached).

v7 additions over v6:
- NCH is a compile-time constant (2 or 3); batch padded to 8*NCH*CW.
- startup: first 2 steps' x/q DMA'd in a small transfer ahead of the
  block-0 bulk so step 0 isn't gated on the full block.
- NCH=3: bt product runs on GPSIMD to keep DVE under the cycle.
"""

import numpy as np

B, T, D, H = 2048, 200, 64, 64
NCORES = 8
NCH = 2                   # chains per core
CW = 128                  # columns per chain
BPAD = NCORES * NCH * CW  # padded batch
KBLK = 8

_CACHE = {}


def _build(ws):
    """ws: tuple of per-step live widths (len = number of steps)."""
    key = ("nc", NCH, CW, ws)
    if key in _CACHE:
        return _CACHE[key]

    from contextlib import ExitStack
    import concourse.tile as tile
    from concourse import bacc, mybir

    TS = len(ws)
    CBLK = (TS - 1) // KBLK         # last block with compute steps

    f32 = mybir.dt.float32
    bf16 = mybir.dt.float16
    ALU = mybir.AluOpType
    AF = mybir.ActivationFunctionType

    nc = bacc.Bacc("TRN2", target_bir_lowering=False, debug=False,
                   enable_asserts=False, num_devices=NCORES)

    XD = nc.dram_tensor("x", [NCH, D, T, CW], bf16, kind="ExternalInput").ap()
    QD = nc.dram_tensor("q", [NCH, H, T, CW], bf16, kind="ExternalInput").ap()
    WPK = nc.dram_tensor("wpk", [D, 2 * H + 2 * H + H + H], bf16, kind="ExternalInput").ap()
    BPK = nc.dram_tensor("bpk", [2 * H, 3], f32, kind="ExternalInput").ap()
    OUT = nc.dram_tensor("out", [NCH, H, T, CW], bf16, kind="ExternalOutput").ap()

    with tile.TileContext(nc) as tc:
        with ExitStack() as ctx:
            consts = ctx.enter_context(tc.tile_pool(name="consts", bufs=1))
            state = ctx.enter_context(tc.tile_pool(name="state", bufs=1))
            gpoolG = ctx.enter_context(tc.tile_pool(name="gatesG", bufs=165))
            gpoolS = ctx.enter_context(tc.tile_pool(name="gatesS", bufs=165))
            tpool = ctx.enter_context(tc.tile_pool(name="tmp", bufs=6))
            rpool = ctx.enter_context(tc.tile_pool(name="rh", bufs=8))
            ps_zg = ctx.enter_context(tc.tile_pool(name="zg", bufs=2, space="PSUM"))
            ps_zc = ctx.enter_context(tc.tile_pool(name="zc", bufs=2, space="PSUM"))

            # ---- shared constants ----
            wpk_sb = consts.tile([D, 6 * H], bf16, tag="wpk")
            nc.sync.dma_start(out=wpk_sb[:], in_=WPK[:])
            bpk_sb = consts.tile([2 * H, 3], f32, tag="bpk")
            nc.sync.dma_start(out=bpk_sb[:], in_=BPK[:])

            # ---- per-chain staged/rotating tiles ----
            xst = [[state.tile([D, KBLK, CW], bf16, tag=f"xst{c}_{j}", name=f"xst{c}_{j}")
                    for j in range(2)] for c in range(NCH)]
            hq = [[state.tile([2 * H, KBLK, CW], bf16, tag=f"hq{c}_{j}", name=f"hq{c}_{j}")
                    for j in range(2)] for c in range(NCH)]

            w1x_sb = wpk_sb[:, 0:2 * H]
            w1h_sb = wpk_sb[:, 2 * H:4 * H]
            w2x_sb = wpk_sb[:, 4 * H:5 * H]
            w2h_sb = wpk_sb[:, 5 * H:6 * H]
            bg_sb = bpk_sb[:, 0:1]
            bc_sb = bpk_sb[0:H, 1:2]
            # dummy sigmoid: hoists the ACT table load off the first chain step
            scr = consts.tile([1, 2], bf16, tag="scr")
            scr3 = consts.tile([1, 2], bf16, tag="scr3")
            nc.scalar.activation(scr[:], bpk_sb[0:1, 0:2], AF.Sigmoid)

            ones_hi = [consts.tile([2 * H, CW], bf16, tag=f"oneshi{c}", name=f"oneshi{c}") for c in range(NCH)]
            half_bt = [consts.tile([H, CW], bf16, tag=f"halfbt{c}", name=f"halfbt{c}") for c in range(NCH)]
            half_m = [consts.tile([H, CW], bf16, tag=f"halfm{c}", name=f"halfm{c}") for c in range(NCH)]
            for c in range(NCH):
                nc.gpsimd.memset(ones_hi[c][H:, :], 1.0)
                nc.vector.memset(half_bt[c][:], 0.5)
                nc.vector.memset(half_m[c][:], 0.5)

            hst = [state.tile([2 * H, 2, CW], bf16, tag=f"hst{c}", name=f"hst{c}")
                   for c in range(NCH)]
            btt = [[state.tile([H, CW], bf16, tag=f"btt{c}_{j}", name=f"btt{c}_{j}")
                    for j in range(2)] for c in range(NCH)]
            mtt = [[state.tile([H, CW], bf16, tag=f"mtt{c}_{j}", name=f"mtt{c}_{j}")
                    for j in range(4)] for c in range(NCH)]

            nc.gpsimd.dma_start(out=xst[0][0][:, :, :], in_=XD[0, :, 0:KBLK, :])
            for c in range(1, NCH):
                nc.gpsimd.dma_start(out=xst[c][0][:, :, :], in_=XD[c, :, 0:KBLK, :])
            for c in range(NCH):
                nc.sync.dma_start(out=hq[c][0][H:, :, :], in_=QD[c, :, 0:KBLK, :])
            for c in range(NCH):
                nc.vector.memset(hq[c][0][0:H, 0, :], 0.0)

            # DMA/OUT slot schedule within each 8-step block, per chain
            if NCH == 3:
                x_slot = {0: 0, 1: 1, 2: 2}                      # prefetch x blk+1
                q_slot = {0: 3, 1: 4, 2: 5}
                o_slot = {0: 6, 1: 7, 2: 5}
            else:
                x_slot = {c: 2 * c for c in range(NCH)}
                q_slot = {c: 2 * c + 1 for c in range(NCH)}
                o_slot = {c: 4 + 2 * c for c in range(NCH)}
            bt_engine = nc.gpsimd if NCH >= 3 else nc.vector

            last_out_j = [-1 for _ in range(NCH)]   # last OUT block written per chain

            for t in range(TS):
                W = ws[t]
                blk, ci = divmod(t, KBLK)
                pb = blk % 2
                for c in range(NCH):
                    # spread the block DMAs across the block's steps
                    if ci == x_slot[c] and blk + 1 <= CBLK:
                        nc.sync.dma_start(out=xst[c][(blk + 1) % 2][:, :, :],
                                          in_=XD[c, :, (blk + 1) * KBLK:(blk + 2) * KBLK, :])
                    if ci == q_slot[c] and blk + 1 <= CBLK:
                        nc.sync.dma_start(out=hq[c][(blk + 1) % 2][H:, :, :],
                                          in_=QD[c, :, (blk + 1) * KBLK:(blk + 2) * KBLK, :])
                    if ci == 6 and blk + 1 <= CBLK:
                        scr2 = tpool.tile([1, 2], bf16, tag=f"scr2{c}", name=f"scr2{c}")
                        nc.vector.tensor_copy(scr2[:], hq[c][(blk + 1) % 2][H:H + 1, 0, 0:2])
                    if ci == o_slot[c] and blk >= 1:
                        j = blk - 1
                        nc.sync.dma_start(
                            out=OUT[c, :, j * KBLK:(j + 1) * KBLK - 1, :],
                            in_=hq[c][j % 2][0:H, 1:KBLK, :])
                        nc.sync.dma_start(
                            out=OUT[c, :, (j + 1) * KBLK - 1:(j + 1) * KBLK, :],
                            in_=hq[c][(j + 1) % 2][0:H, 0:1, :])
                        last_out_j[c] = j

                    if t == 0:
                        h_tilde_prev = ones_hi[c][H:, 0:W]
                        bt_prev = half_bt[c][:, 0:W]
                        m_prev = half_m[c][:, 0:W]
                    else:
                        h_tilde_prev = hst[c][H:, (t - 1) % 2, 0:W]
                        bt_prev = btt[c][(t - 1) % 2][:, 0:W]
                        m_prev = mtt[c][(t - 1) % 4][:, 0:W]

                    # gate preactivation
                    zg = ps_zg.tile([2 * H, CW], f32, tag=f"zg{c}", name=f"zg{c}")
                    nc.tensor.matmul(zg[:, 0:W], lhsT=w1x_sb,
                                     rhs=xst[c][pb][:, ci, 0:W],
                                     start=True, stop=(t == 0))
                    if t > 0:
                        nc.tensor.matmul(zg[:, 0:W], lhsT=w1h_sb, rhs=bt_prev,
                                         start=False, stop=False)
                        nc.tensor.matmul(zg[:, 0:W], lhsT=w1h_sb, rhs=m_prev,
                                         start=False, stop=True)

                    G = gpoolG.tile([2 * H, CW], bf16, tag=f"G{c}", name=f"G{c}")
                    nc.scalar.activation(G[:, 0:W], zg[:, 0:W], AF.Sigmoid,
                                         bias=bg_sb if t > 0 else bpk_sb[:, 2:3])

                    # DVE: rh (chain), uh, a2; bt on Pool when NCH>=3
                    ru = rpool.tile([2 * H, CW], bf16, tag=f"ru{c}", name=f"ru{c}")
                    nc.vector.tensor_tensor(ru[:, 0:W], hq[c][pb][:, ci, 0:W],
                                            G[:, 0:W], op=ALU.mult)
                    a2 = tpool.tile([H, CW], bf16, tag=f"a2{c}", name=f"a2{c}")
                    nc.vector.tensor_scalar(a2[:, 0:W], ru[H:, 0:W], -2.0, 2.0,
                                            op0=ALU.mult, op1=ALU.add)
                    bt_cur = btt[c][t % 2]
                    bt_engine.tensor_tensor(bt_cur[:, 0:W], ru[H:, 0:W], h_tilde_prev,
                                            op=ALU.mult)

                    # candidate
                    zc = ps_zc.tile([H, CW], f32, tag=f"zc{c}", name=f"zc{c}")
                    nc.tensor.matmul(zc[:, 0:W], lhsT=w2x_sb,
                                     rhs=xst[c][pb][:, ci, 0:W],
                                     start=True, stop=(t == 0))
                    if t > 0:
                        nc.tensor.matmul(zc[:, 0:W], lhsT=w2h_sb, rhs=ru[0:H, 0:W],
                                         start=False, stop=True)
                    s = gpoolS.tile([H, CW], bf16, tag=f"s{c}", name=f"s{c}")
                    nc.scalar.activation(s[:, 0:W], zc[:, 0:W], AF.Sigmoid, bias=bc_sb,
                                         scale=2.0)

                    # m (chain, DVE); h~' and h' follow
                    m_cur = mtt[c][t % 4]
                    nc.vector.tensor_tensor(m_cur[:, 0:W], a2[:, 0:W], s[:, 0:W],
                                            op=ALU.mult)
                    nc.vector.tensor_tensor(hst[c][H:, t % 2, 0:W], bt_cur[:, 0:W],
                                            m_cur[:, 0:W], op=ALU.add)
                    nblk2, nci = divmod(t + 1, KBLK)
                    nc.vector.tensor_scalar(hq[c][nblk2 % 2][0:H, nci, 0:W],
                                            hst[c][H:, t % 2, 0:W],
                                            -1.0, None, op0=ALU.add)
                    if c == NCH - 1 and t >= TS - 44 and t % 8 == 5:
                        # advance ACT's DVE-sem frontier: sigmoid WAR conds
                        # on reused G/s tiles get statically subsumed
                        nc.scalar.activation(scr3[:], mtt[NCH - 1][(t - 2) % 4][0:1, 0:2],
                                             AF.Sigmoid)

            # Epilogue: flush OUT blocks not yet written.  h_p lives in
            # hq[(p+1)//8 % 2] slot (p+1)%8; positions p = 0..TS-1.
            for c in range(NCH):
                for j in range(last_out_j[c] + 1, CBLK + 1):
                    hi_p = min((j + 1) * KBLK - 2, TS - 1)       # last p with slot in block j
                    nslots = hi_p - j * KBLK + 1
                    if nslots >= 1:
                        nc.sync.dma_start(
                            out=OUT[c, :, j * KBLK:j * KBLK + nslots, :],
                            in_=hq[c][j % 2][0:H, 1:1 + nslots, :])
                    p = (j + 1) * KBLK - 1                        # slot 0 of block j+1
                    if p <= TS - 1:
                        nc.sync.dma_start(
                            out=OUT[c, :, p:p + 1, :],
                            in_=hq[c][(j + 1) % 2][0:H, 0:1, :])

    nc.compile()
    _CACHE[key] = nc
    return nc


def _prep_shared(gate_kernel, gate_bias, cand_kernel, cand_bias):
    bf = np.dtype(np.float16)
    gk = np.asarray(gate_kernel, np.float32)
    gb = np.asarray(gate_bias, np.float32)
    ck = np.asarray(cand_kernel, np.float32)
    cb = np.asarray(cand_bias, np.float32)
    wpk = np.concatenate([gk[:D], gk[D:], ck[:D], ck[D:]], axis=1).astype(bf)  # [64, 384]
    bg = (gb - gk[D:].sum(axis=0)).astype(np.float32)
    bpk = np.zeros((2 * H, 3), np.float32)
    bpk[:, 0] = bg
    bpk[:H, 1] = 2.0 * cb
    bpk[:, 2] = gb
    return np.ascontiguousarray(wpk), bpk


def _schedule(seq_len):
    """Sorted column assignment + per-step width schedule.

    Returns (cols, ws): cols[k, c, i] = padded-batch row for core k,
    chain c, column i (rows >= B are zero padding);  ws[t] = compiled
    live width at step t.
    """
    seq = np.zeros(BPAD, np.int64)
    seq[:B] = np.asarray(seq_len, np.int64)
    order = np.argsort(-seq, kind="stable")          # rank r -> padded row
    cols = np.empty((NCORES, NCH, CW), np.int64)
    for k in range(NCORES):
        for c in range(NCH):
            cols[k, c] = order[k + NCORES * c + NCORES * NCH * np.arange(CW)]
    sdesc = seq[order]
    ts = max(int(sdesc[0]), 1)                       # steps needed = max seq_len
    alive = np.array([(sdesc > t).sum() for t in range(ts)], np.int64)
    ws = np.minimum(CW, np.maximum(2, -(-alive // (NCORES * NCH))))
    return cols, tuple(int(w) for w in ws)


def _run(inputs, trace=False):
    from concourse.bass_utils import run_bass_kernel_spmd

    bf = np.dtype(np.float16)
    rnn_input = np.asarray(inputs["rnn_input"], np.float32)
    seq_len = np.asarray(inputs["sequence_length"], np.int32)
    att = np.asarray(inputs["att_score"], np.float32)
    wpk, bpk = _prep_shared(
        inputs["gate_kernel"], inputs["gate_bias"],
        inputs["cand_kernel"], inputs["cand_bias"])

    cols, ws = _schedule(seq_len)
    nc = _build(ws)

    rnn_pad = np.zeros((BPAD, T, D), np.float32)
    rnn_pad[:B] = rnn_input
    q_pad = np.ones((BPAD, T), np.float32)
    q_pad[:B] = 1.0 - att[:, :, 0]

    in_maps = []
    for k in range(NCORES):
        xi = np.empty((NCH, D, T, CW), bf)
        qi = np.empty((NCH, H, T, CW), bf)
        for c in range(NCH):
            rows = cols[k, c]
            xi[c] = rnn_pad[rows].transpose(2, 1, 0).astype(bf)
            q = q_pad[rows].T.astype(np.float32)               # [T,CW]
            qi[c] = np.broadcast_to(q[None], (H, T, CW)).astype(bf)
        in_maps.append({"x": np.ascontiguousarray(xi),
                        "q": np.ascontiguousarray(qi),
                        "wpk": wpk, "bpk": bpk})

    res = run_bass_kernel_spmd(nc, in_maps, core_ids=list(range(NCORES)), trace=trace)

    out = np.zeros((B, T, H), np.float32)
    t_idx = np.arange(T, dtype=np.int32)
    for k in range(NCORES):
        y = np.asarray(res.results[k]["out"], np.float32)   # [NCH,H,T,CW]
        for c in range(NCH):
            rows = cols[k, c]
            live = rows < B
            r = rows[live]
            hi = y[c].transpose(2, 1, 0)[live]              # [nlive,T,H]
            mask = (t_idx[None, :] < seq_len[r, None]).astype(np.float32)
            out[r] = hi * mask[:, :, None]
    return out, res


def kernel(**inputs):
    out, _ = _run(inputs)
    return out
